# revision 1
# baseline (speedup 1.0000x reference)
"""Trainium2 Bass kernel for a 2-layer GAT (nn_GAT_83382495084588).

Distribution (8 NeuronCores, one chip, pure SPMD — one program, per-core data):
  - dst-node sharding: nodes are lex-sorted by (a, b) = per-node counts of
    src's in table-half A/B, chunked into 128-node tiles, snake-dealt to cores
    so every core has the same per-round slot schedule (DA[r], DB[r]).
  - Phase 0 (replicated): every core computes the full Z1 table
    [feat1.fp16 | el1.f32 | er1.f32] (h @ [W1 | W1@al1_bd | W1@ar1_bd], fp32r
    matmul) into its own DRAM — cheaper than a 51MB AllGather of feat1.
  - Layer-1 edge phase: per (round, half) one dma_gather (int16 idx) fetches
    768B packed rows for all slots incl. a leading self-row (supplies er of
    the dst node itself; the opposite half's self-row points at a zero row).
    Segment softmax over padded slots (additive -1e30 mask, no max-subtract;
    |e| <= ~4 verified), then per-slot msg = feat*alpha on DVE (out fp32r)
    accumulated with identity-matmuls into PSUM.
  - Inter-layer: each core scatters its compact z2 rows [feat2.fp16|el2|er2]
    (136B) into an A|B-ordered shard via per-round indirect row scatters, two
    AllGathers (A and B halves, ~3.4MB each) exchange them, then a DRAM->DRAM
    repack pads rows to 256B for gathering.
  - Layer-2 edge phase mirrors layer 1 (1 head, 64 dims, er2 is core-local).

kernel(**inputs) takes the full unsharded inputs and returns the full
(50000, 64) output; host numpy does only sharding/index prep + unshuffle.
"""

import os
import sys
from dataclasses import dataclass, field

import numpy as np

for _p in ("/opt/trn_rl_repo", "/root/.axon_site/_ro/trn_rl_repo"):
    if os.path.isdir(_p) and _p not in sys.path:
        sys.path.append(_p)

import concourse.bacc as bacc
import concourse.bass as bass
import concourse.mybir as mybir
import concourse.tile as tile
from concourse.bass import IndirectOffsetOnAxis
from concourse.bass_utils import run_bass_kernel_spmd

F32 = mybir.dt.float32
F32R = mybir.dt.float32r
F16 = mybir.dt.float16
I32 = mybir.dt.int32
I16 = mybir.dt.int16
AF = mybir.ActivationFunctionType
OP = mybir.AluOpType

P = 128
NCC = 8
PHASES = int(os.environ.get("GAT_PHASES", "4"))
L1SUB = int(os.environ.get("GAT_L1SUB", "0"))
NEG_SLOPE = 0.2
F32R_INPUTS = {"htiles", "W1ext", "W2ext", "identr"}
I16_INPUTS = {"gidx1", "gidx2"}


@dataclass
class Prob:
    N: int
    IN_DIM: int
    H1: int
    HID: int
    OUT_DIM: int
    rounds: int = 0
    DA: list = field(default_factory=list)
    DB: list = field(default_factory=list)
    # derived
    C1: int = 0        # H1*HID
    Z1W: int = 0       # L1 table width in f32 words (768B -> 192)
    Z2W: int = 0       # L2 gather-table width in f32 words (256B -> 64)
    Z2C: int = 0       # compact z2 row width in f32 words (34)
    NT: int = 0        # rounds * P (nodes per core)
    HALF: int = 0      # rows per table half
    SD: int = 0        # sum(DA) + sum(DB)
    l2_groups: list = field(default_factory=list)  # list of round-ranges

    def finish(self):
        self.C1 = self.H1 * self.HID
        self.Z1W = self.C1 // 2 + 2 * self.H1 + 48   # fp16-packed feat + el + er + pad
        self.Z2W = 64
        self.Z2C = 64
        self.NT = self.rounds * P
        self.HALF = NCC * self.NT // 2
        self.SD = int(sum(self.DA) + sum(self.DB))
        return self


def wrap16(flat_idx):
    """dma_gather idx layout: flat i -> [i%16, i//16], replicated to 128 rows."""
    n = len(flat_idx)
    S = max(1, (n + 15) // 16)
    t = np.zeros((16, S), np.int16)
    ii = np.arange(n)
    t[ii % 16, ii // 16] = flat_idx
    return np.tile(t, (8, 1))


def prep_all(inputs, pr: Prob):
    rng = np.random.default_rng(12345)
    src = np.asarray(inputs["src"]).astype(np.int64)
    dst = np.asarray(inputs["dst"]).astype(np.int64)
    h = np.asarray(inputs["h"], dtype=np.float32)
    W1 = np.asarray(inputs["W1"], dtype=np.float32)
    al1 = np.asarray(inputs["al1"], dtype=np.float32)
    ar1 = np.asarray(inputs["ar1"], dtype=np.float32)
    b1 = np.asarray(inputs["b1"], dtype=np.float32)
    W2 = np.asarray(inputs["W2"], dtype=np.float32)
    al2 = np.asarray(inputs["al2"], dtype=np.float32)
    ar2 = np.asarray(inputs["ar2"], dtype=np.float32)
    b2 = np.asarray(inputs["b2"], dtype=np.float32)
    N = pr.N

    deg = np.bincount(dst, minlength=N)
    NT_G = ((N + NCC * P - 1) // (NCC * P)) * (NCC * P)   # padded global nodes
    rounds = NT_G // (NCC * P)
    n_dummy = NT_G - N

    # --- initial random half designation over real nodes ---
    desA = np.zeros(N, bool)
    desA[rng.permutation(N)[:N // 2]] = True

    # --- (a, b) counts and lex sort ---
    cA = np.zeros(N, np.int64)
    np.add.at(cA, dst, desA[src].astype(np.int64))
    cB = deg - cA
    order = np.lexsort((cB, cA))
    gnodes = np.concatenate([order, np.full(n_dummy, -1, np.int64)])

    # --- snake deal to cores ---
    assign = np.zeros((NCC, rounds), np.int64)
    for r in range(rounds):
        for c in range(NCC):
            assign[c, r] = r * NCC + (c if r % 2 == 0 else NCC - 1 - c)
    core_nodes = np.zeros((NCC, rounds * P), np.int64)
    for c in range(NCC):
        for r in range(rounds):
            t = assign[c, r]
            core_nodes[c, r * P:(r + 1) * P] = gnodes[t * P:(t + 1) * P]

    # --- rebalance: each core must own exactly NT/2 A-designated rows.
    # dummies are free to designate; then flip real nodes if still uneven.
    halfNT = rounds * P // 2
    desA_d = {}   # designation of dummy slots per (core, pos)
    for c in range(NCC):
        nodes = core_nodes[c]
        real = nodes[nodes >= 0]
        nA = int(desA[real].sum())
        dummies = np.where(nodes < 0)[0]
        need = halfNT - nA
        take = max(0, min(len(dummies), need))
        for j, posi in enumerate(dummies):
            desA_d[(c, int(posi))] = j < take
        need -= take
        if need > 0:      # designate more real B -> A
            bsel = real[~desA[real]]
            desA[bsel[:need]] = True
        elif need < 0:    # demote some real A -> B
            asel = real[desA[real]]
            desA[asel[:(-need)]] = False

    # recompute (a, b) under final designation (tiles unchanged)
    cA = np.zeros(N, np.int64)
    np.add.at(cA, dst, desA[src].astype(np.int64))
    cB = deg - cA
    DA = np.zeros(rounds, np.int64)
    DB = np.zeros(rounds, np.int64)
    for r in range(rounds):
        sel = gnodes[r * NCC * P:(r + 1) * NCC * P]
        realr = sel[sel >= 0]
        DA[r] = max(1, int(cA[realr].max()) if len(realr) else 1)
        DB[r] = max(1, int(cB[realr].max()) if len(realr) else 1)

    pr.rounds = rounds
    pr.DA = [int(x) for x in DA]
    pr.DB = [int(x) for x in DB]
    pr.finish()

    # --- L1 table order: all A rows then all B rows (core/position major) ---
    # row content for dummies is zero (h row = 0).
    pos1 = np.full(N, -1, np.int64)          # node -> L1 table row
    h_table = np.zeros((NCC * pr.NT, pr.IN_DIM), np.float32)
    ra, rb = 0, pr.HALF
    # also L2 shard ranks: node -> (core, rank within core's A or B shard part)
    shard_rank = np.full(NCC * pr.NT, -1, np.int64)   # per (c, pos)
    pos2 = np.full(N, -1, np.int64)          # node -> L2 table row (A:0..HALF)
    for c in range(NCC):
        ca = cb = 0
        for posi in range(pr.NT):
            n = core_nodes[c, posi]
            if n >= 0:
                isa = bool(desA[n])
            else:
                isa = desA_d.get((c, posi), False)
            if isa:
                if n >= 0:
                    pos1[n] = ra
                    h_table[ra] = h[n]
                    pos2[n] = c * halfNT + ca
                shard_rank[c * pr.NT + posi] = ca
                ra += 1
                ca += 1
            else:
                if n >= 0:
                    pos1[n] = rb
                    h_table[rb] = h[n]
                    pos2[n] = NCC * halfNT + c * halfNT + cb
                shard_rank[c * pr.NT + posi] = halfNT + cb
                rb += 1
                cb += 1
    assert ra == pr.HALF and rb == 2 * pr.HALF

    # --- CSR by dst, split per node into A-edges / B-edges ---
    sort = np.argsort(dst, kind="stable")
    s_src = src[sort]
    starts = np.zeros(N + 1, np.int64)
    np.cumsum(deg, out=starts[1:])

    offs = np.zeros(rounds + 1, np.int64)
    np.cumsum(np.array(pr.DA) + np.array(pr.DB), out=offs[1:])

    per_core = []
    for c in range(NCC):
        nodes = core_nodes[c]
        mask = np.full((P, pr.SD), np.float32(-1e30), np.float32)
        gi1 = []   # int16 idx stream for L1 gathers (per round: A then B unit)
        gi2 = []   # for L2
        scat = np.zeros((P, rounds), np.int32)
        selfA = np.zeros((P, rounds), np.int32)
        for r in range(rounds):
            da, db = pr.DA[r], pr.DB[r]
            off = offs[r]
            iA1 = np.zeros((1 + da, P), np.int64)   # [slot, partition]
            iB1 = np.zeros((1 + db, P), np.int64)
            iA2 = np.zeros((da, P), np.int64)
            iB2 = np.zeros((db, P), np.int64)
            for p in range(P):
                n = nodes[r * P + p]
                scat[p, r] = shard_rank[c * pr.NT + r * P + p]
                if n < 0:
                    mask[p, off] = 0.0   # one live A-slot (row 0) keeps denom > 0
                    continue
                if desA[n]:
                    iA1[0, p] = pos1[n]
                    selfA[p, r] = 1
                else:
                    iB1[0, p] = pos1[n] - pr.HALF
                srcs = s_src[starts[n]:starts[n + 1]]
                sa = srcs[desA[srcs]]
                sb = srcs[~desA[srcs]]
                iA1[1:1 + len(sa), p] = pos1[sa]
                iB1[1:1 + len(sb), p] = pos1[sb] - pr.HALF
                iA2[:len(sa), p] = pos2[sa]
                iB2[:len(sb), p] = pos2[sb] - NCC * halfNT
                mask[p, off:off + len(sa)] = 0.0
                mask[p, off + da:off + da + len(sb)] = 0.0
            gi1.append(iA1.reshape(-1))
            gi1.append(iB1.reshape(-1))
            gi2.append(iA2.reshape(-1))
            gi2.append(iB2.reshape(-1))
        gidx1 = wrap16(np.concatenate(gi1))
        gidx2 = wrap16(np.concatenate(gi2))
        per_core.append(dict(gidx1=gidx1.astype(np.int16),
                             gidx2=gidx2.astype(np.int16),
                             mask=mask, scat=scat, selfA=selfA))

    # --- htiles for phase 0 (L1-table order, transposed, tiled) ---
    T0 = (NCC * pr.NT) // P
    ht = h_table.reshape(T0, P, 2, P).transpose(3, 0, 2, 1).reshape(P, T0 * 2, P)
    htiles = np.ascontiguousarray(ht)

    # --- extended weights (f64 host precompute) ---
    H1n, HID = pr.H1, pr.HID
    al_bd = np.zeros((pr.C1, H1n), np.float64)
    ar_bd = np.zeros((pr.C1, H1n), np.float64)
    for hh in range(H1n):
        al_bd[hh * HID:(hh + 1) * HID, hh] = al1[hh].astype(np.float64)
        ar_bd[hh * HID:(hh + 1) * HID, hh] = ar1[hh].astype(np.float64)
    W1f = W1.astype(np.float64)
    W1ext = np.concatenate([W1, (W1f @ al_bd).astype(np.float32),
                            (W1f @ ar_bd).astype(np.float32)], axis=1)
    W2f = W2.astype(np.float64)
    W2ext = np.concatenate(
        [W2, (W2f @ al2.astype(np.float64).reshape(-1, 1)).astype(np.float32),
         (W2f @ ar2.astype(np.float64).reshape(-1, 1)).astype(np.float32)], axis=1)

    shared = dict(
        htiles=htiles,
        W1ext=np.ascontiguousarray(W1ext),
        W2ext=np.ascontiguousarray(W2ext),
        identr=np.eye(P, dtype=np.float32),
        identf=np.eye(P, dtype=np.float32),
        b1rep=np.broadcast_to(b1, (P, pr.C1)).copy(),
        b2rep=np.broadcast_to(b2, (P, pr.OUT_DIM)).copy(),
    )
    in_maps = []
    for c in range(NCC):
        m = dict(shared)
        m.update(per_core[c])
        in_maps.append(m)
    sched = dict(core_nodes=core_nodes, rounds=rounds)
    return sched, in_maps


def build_kernel_fn(pr: Prob):
    rounds, DA, DB = pr.rounds, pr.DA, pr.DB
    C1, Z1W, Z2W, Z2C = pr.C1, pr.Z1W, pr.Z2W, pr.Z2C
    H1, HID, OUT = pr.H1, pr.HID, pr.OUT_DIM
    T0 = (NCC * pr.NT) // P
    HALF = pr.HALF
    halfNT = pr.NT // 2
    F16C = C1 // 2            # feat fp16 packed into f32 words
    EL0 = F16C                # el word offset in Z1 row
    ER0 = F16C + H1
    offs = np.zeros(rounds + 1, np.int64)
    np.cumsum(np.array(DA) + np.array(DB), out=offs[1:])
    # idx stream offsets (in int16-wrapped columns: 1 col = 16 idx)
    gi1_off, gi2_off = [0], [0]
    for r in range(rounds):
        gi1_off.append(gi1_off[-1] + ((1 + DA[r]) + (1 + DB[r])) * 8)
        gi2_off.append(gi2_off[-1] + (DA[r] + DB[r]) * 8)

    def kern(tc: tile.TileContext, outs, ins):
        nc = tc.nc

        Z1 = nc.dram_tensor("Z1d", [2 * HALF, Z1W], F32)
        z2shard = nc.dram_tensor("z2shardd", [pr.NT, Z2C], F32)
        Z2 = nc.dram_tensor("Z2d", [2 * HALF, Z2W], F32, addr_space="Shared")

        with (
            tc.tile_pool(name="const", bufs=1) as cpool,
            tc.tile_pool(name="big", bufs=1) as big,
        ):
            # ---- constants ----
            w1e = cpool.tile([P, 2, C1 + 2 * H1], F32R)
            for c in range(2):
                nc.sync.dma_start(w1e[:, c, :], ins["W1ext"][c * P:(c + 1) * P, :])
            w2e = cpool.tile([P, 2, OUT + 2], F32R)
            for c in range(2):
                nc.sync.dma_start(w2e[:, c, :], ins["W2ext"][c * P:(c + 1) * P, :])
            ident = cpool.tile([P, P], F32R)
            nc.sync.dma_start(ident[:], ins["identr"][:, :])
            identf = cpool.tile([P, P], F32)
            nc.sync.dma_start(identf[:], ins["identf"][:, :])
            b1r = cpool.tile([P, C1], F32)
            nc.sync.dma_start(b1r[:], ins["b1rep"][:, :])
            b2r = cpool.tile([P, OUT], F32)
            nc.sync.dma_start(b2r[:], ins["b2rep"][:, :])
            gidx1 = cpool.tile([P, gi1_off[-1]], I16)
            nc.sync.dma_start(gidx1[:], ins["gidx1"][:, :])
            gidx2 = cpool.tile([P, gi2_off[-1]], I16)
            nc.sync.dma_start(gidx2[:], ins["gidx2"][:, :])
            maskt = cpool.tile([P, pr.SD], F32)
            nc.sync.dma_start(maskt[:], ins["mask"][:, :])
            scatt = cpool.tile([P, rounds], I32)
            nc.sync.dma_start(scatt[:], ins["scat"][:, :])
            selfat = cpool.tile([P, rounds], I32)
            nc.sync.dma_start(selfat[:], ins["selfA"][:, :])

            h1all = big.tile([P, rounds, C1], F32)
            z2all = big.tile([P, rounds, Z2C], F32)
            nc.vector.memset(z2all[:, :, OUT // 2 + 2:Z2C], 0.0)

            if PHASES < 1:
                return
            # ---- phase 0: Z1 = h @ [W1|W1al|W1ar] (replicated; fp16 feat) ----
            with (
                tc.tile_pool(name="p0h", bufs=4) as p0h,
                tc.tile_pool(name="p0ps", bufs=4, space="PSUM") as p0ps,
                tc.tile_pool(name="p0z", bufs=4) as p0z,
            ):
                for t in range(T0):
                    ht = p0h.tile([P, 2, P], F32R)
                    nc.sync.dma_start(ht[:], ins["htiles"][:, 2 * t:2 * t + 2, :])
                    zps = p0ps.tile([P, C1 + 2 * H1], F32)
                    nc.tensor.matmul(zps[:], lhsT=ht[:, 0, :],
                                     rhs=w1e[:, 0, :], start=True, stop=False)
                    nc.tensor.matmul(zps[:], lhsT=ht[:, 1, :],
                                     rhs=w1e[:, 1, :], start=False, stop=True)
                    zsb = p0z.tile([P, Z1W], F32)
                    nc.vector.tensor_copy(
                        zsb[:, 0:F16C].bitcast(F16), zps[:, 0:C1])
                    nc.vector.tensor_copy(
                        zsb[:, EL0:ER0 + H1], zps[:, C1:C1 + 2 * H1])
                    nc.vector.memset(zsb[:, ER0 + H1:Z1W], 0.0)
                    nc.sync.dma_start(Z1[t * P:(t + 1) * P, :], zsb[:])

            if PHASES < 2:
                return
            # ---- layer-1 edge phase ----
            with (
                tc.tile_pool(name="fg", bufs=2) as fgp,
                tc.tile_pool(name="al", bufs=3) as alp,
                tc.tile_pool(name="sm", bufs=4) as smp,
                tc.tile_pool(name="msg", bufs=4) as msgp,
                tc.tile_pool(name="l1ps", bufs=2, space="PSUM") as l1ps,
                tc.tile_pool(name="ep", bufs=3) as epp,
            ):
                for r in range(rounds):
                    da, db = DA[r], DB[r]
                    off = int(offs[r])
                    c0 = gi1_off[r]
                    cab = (1 + da) * 8
                    idxA = fgp.tile([P, (1 + da) * 8], I16, tag="idxA")
                    nc.sync.dma_start(idxA[:], ins["gidx1"][:, c0:c0 + cab])
                    idxB = fgp.tile([P, (1 + db) * 8], I16, tag="idxB")
                    nc.sync.dma_start(idxB[:],
                                      ins["gidx1"][:, c0 + cab:c0 + cab + (1 + db) * 8])
                    gA = fgp.tile([P, 1 + da, Z1W], F32, tag="gA")
                    nc.gpsimd.dma_gather(gA[:], Z1[0:HALF, :], idxA[:],
                                         (1 + da) * P, (1 + da) * P, Z1W,
                                         single_packet=False)
                    gB = fgp.tile([P, 1 + db, Z1W], F32, tag="gB")
                    nc.gpsimd.dma_gather(gB[:], Z1[HALF:2 * HALF, :], idxB[:],
                                         (1 + db) * P, (1 + db) * P, Z1W,
                                         single_packet=False)
                    if L1SUB == 1:
                        continue
                    # er = A-half self-row if node designated A else B-half's
                    er = alp.tile([P, H1], F32, tag="er")
                    nc.vector.tensor_copy(er[:], gB[:, 0, ER0:ER0 + H1])
                    nc.vector.copy_predicated(
                        er[:], selfat[:, r:r + 1].to_broadcast((P, H1)),
                        gA[:, 0, ER0:ER0 + H1])
                    # e = lrelu(el + er) + mask over the da+db edge slots
                    ew = alp.tile([P, da + db, H1], F32, tag="ew")
                    tmp = alp.tile([P, da + db, H1], F32, tag="tmp")
                    for (s0, dd_, gt) in ((0, da, gA), (da, db, gB)):
                        nc.vector.tensor_tensor(
                            out=ew[:, s0:s0 + dd_, :],
                            in0=gt[:, 1:1 + dd_, EL0:EL0 + H1],
                            in1=er[:, None, :].to_broadcast((P, dd_, H1)),
                            op=OP.add)
                    nc.vector.tensor_scalar_mul(tmp[:], ew[:], NEG_SLOPE)
                    nc.vector.tensor_tensor(out=ew[:], in0=ew[:], in1=tmp[:],
                                            op=OP.max)
                    nc.vector.tensor_tensor(
                        out=ew[:], in0=ew[:],
                        in1=maskt[:, off:off + da + db, None]
                            .to_broadcast((P, da + db, H1)),
                        op=OP.add)
                    nc.scalar.activation(out=ew[:], in_=ew[:], func=AF.Exp)
                    den = smp.tile([P, H1], F32, tag="den")
                    nc.vector.reduce_sum(
                        out=den[:], in_=ew[:].rearrange("p d h -> p h d"),
                        axis=mybir.AxisListType.X)
                    nc.vector.reciprocal(out=den[:], in_=den[:])
                    nc.vector.tensor_tensor(
                        out=ew[:], in0=ew[:],
                        in1=den[:, None, :].to_broadcast((P, da + db, H1)),
                        op=OP.mult)
                    # msg accumulation
                    ps = l1ps.tile([P, C1], F32)
                    for k in range(da + db):
                        gsl = gA[:, 1 + k, 0:F16C] if k < da \
                            else gB[:, 1 + k - da, 0:F16C]
                        mg = msgp.tile([P, C1], F32R, tag="mg")
                        nc.vector.tensor_tensor(
                            out=mg[:].rearrange("p (h d) -> p h d", h=H1),
                            in0=gsl.bitcast(F16)
                                .rearrange("p (h d) -> p h d", h=H1),
                            in1=ew[:, k, :, None].to_broadcast((P, H1, HID)),
                            op=OP.mult)
                        if L1SUB != 3:
                            nc.tensor.matmul(ps[:], lhsT=ident[:], rhs=mg[:],
                                             start=(k == 0), stop=(k == da + db - 1))
                    if L1SUB == 3:
                        continue
                    # epilogue: h1 = elu(psum + b1)
                    x = epp.tile([P, C1], F32, tag="x")
                    nc.vector.tensor_tensor(out=x[:], in0=ps[:], in1=b1r[:],
                                            op=OP.add)
                    mn = epp.tile([P, C1], F32, tag="mn")
                    nc.vector.tensor_scalar_min(mn[:], x[:], 0.0)
                    exn = epp.tile([P, C1], F32, tag="exn")
                    nc.scalar.activation(out=exn[:], in_=mn[:], func=AF.Exp)
                    nc.vector.tensor_scalar_max(x[:], x[:], 0.0)
                    nc.vector.tensor_tensor(out=h1all[:, r, :], in0=x[:],
                                            in1=exn[:], op=OP.add)
                    nc.vector.tensor_scalar_sub(h1all[:, r, :], h1all[:, r, :], 1.0)

            if PHASES < 3:
                return
            # ---- layer-2 matmul + shard scatter ----
            with (
                tc.tile_pool(name="tps", bufs=2, space="PSUM") as tpsp,
                tc.tile_pool(name="h1t", bufs=3) as h1tp,
                tc.tile_pool(name="z2ps", bufs=2, space="PSUM") as z2psp,
            ):
                for r in range(rounds):
                    tps = tpsp.tile([P, 2, P], F32)
                    for c in range(2):
                        nc.tensor.transpose(out=tps[:, c, :],
                                            in_=h1all[:, r, c * P:(c + 1) * P],
                                            identity=identf[:])
                    h1t = h1tp.tile([P, 2, P], F32R)
                    nc.vector.tensor_copy(h1t[:], tps[:])
                    z2ps = z2psp.tile([P, OUT + 2], F32)
                    for c in range(2):
                        nc.tensor.matmul(z2ps[:], lhsT=h1t[:, c, :],
                                         rhs=w2e[:, c, :],
                                         start=(c == 0), stop=(c == 1))
                    nc.vector.tensor_copy(
                        z2all[:, r, 0:OUT // 2].bitcast(F16), z2ps[:, 0:OUT])
                    nc.vector.tensor_copy(
                        z2all[:, r, OUT // 2:OUT // 2 + 2], z2ps[:, OUT:OUT + 2])
                    nc.gpsimd.indirect_dma_start(
                        out=z2shard[:, :], out_offset=IndirectOffsetOnAxis(
                            ap=scatt[:, r:r + 1], axis=0),
                        in_=z2all[:, r, :], in_offset=None)

            nc.gpsimd.collective_compute(
                "AllGather", OP.bypass, replica_groups=[list(range(NCC))],
                ins=[z2shard[0:halfNT, :]], outs=[Z2[0:HALF, :]])
            nc.gpsimd.collective_compute(
                "AllGather", OP.bypass, replica_groups=[list(range(NCC))],
                ins=[z2shard[halfNT:pr.NT, :]], outs=[Z2[HALF:2 * HALF, :]])

            if PHASES < 4:
                return
            # ---- layer-2 edge phase ----
            EL2 = OUT // 2
            ER2 = OUT // 2 + 1
            MAXDD = max(DA[i] + DB[i] for i in range(rounds))
            out_ap = outs["out"]
            with (
                tc.tile_pool(name="fg2", bufs=3) as fg2p,
                tc.tile_pool(name="al2", bufs=3) as al2p,
                tc.tile_pool(name="msg2", bufs=4) as msg2p,
                tc.tile_pool(name="l2ps", bufs=2, space="PSUM") as l2ps,
                tc.tile_pool(name="ep2", bufs=3) as ep2p,
            ):
                for r in range(rounds):
                    da, db = DA[r], DB[r]
                    off = int(offs[r])
                    dd = da + db
                    c0 = gi2_off[r]
                    idxA = fg2p.tile([P, da * 8], I16, tag="idx2A")
                    nc.sync.dma_start(idxA[:], ins["gidx2"][:, c0:c0 + da * 8])
                    idxB = fg2p.tile([P, db * 8], I16, tag="idx2B")
                    nc.sync.dma_start(idxB[:],
                                      ins["gidx2"][:, c0 + da * 8:c0 + dd * 8])
                    ggA = fg2p.tile([P, da, Z2W], F32, tag="fg2A")
                    nc.gpsimd.dma_gather(ggA[:], Z2[0:HALF, :], idxA[:],
                                         da * P, da * P, Z2W,
                                         single_packet=False)
                    ggB = fg2p.tile([P, db, Z2W], F32, tag="fg2B")
                    nc.gpsimd.dma_gather(ggB[:], Z2[HALF:2 * HALF, :], idxB[:],
                                         db * P, db * P, Z2W,
                                         single_packet=False)
                    ew = al2p.tile([P, MAXDD], F32, tag="ew2")
                    tmp = al2p.tile([P, MAXDD], F32, tag="tmp2")
                    for (s0, dd_, gt) in ((0, da, ggA), (da, db, ggB)):
                        nc.vector.tensor_tensor(
                            out=ew[:, s0:s0 + dd_], in0=gt[:, 0:dd_, EL2],
                            in1=z2all[:, r, ER2:ER2 + 1].to_broadcast((P, dd_)),
                            op=OP.add)
                    nc.vector.tensor_scalar_mul(tmp[:, 0:dd], ew[:, 0:dd],
                                                NEG_SLOPE)
                    nc.vector.tensor_tensor(out=ew[:, 0:dd], in0=ew[:, 0:dd],
                                            in1=tmp[:, 0:dd], op=OP.max)
                    nc.vector.tensor_tensor(out=ew[:, 0:dd], in0=ew[:, 0:dd],
                                            in1=maskt[:, off:off + dd],
                                            op=OP.add)
                    nc.scalar.activation(out=ew[:, 0:dd], in_=ew[:, 0:dd],
                                         func=AF.Exp)
                    den = al2p.tile([P, 1], F32, tag="den2")
                    nc.vector.reduce_sum(out=den[:], in_=ew[:, 0:dd],
                                         axis=mybir.AxisListType.X)
                    nc.vector.reciprocal(out=den[:], in_=den[:])
                    nc.vector.tensor_tensor(
                        out=ew[:, 0:dd], in0=ew[:, 0:dd],
                        in1=den[:].to_broadcast((P, dd)), op=OP.mult)
                    ps = l2ps.tile([P, OUT], F32)
                    for k in range(dd):
                        gsl = ggA[:, k, 0:OUT // 2] if k < da \
                            else ggB[:, k - da, 0:OUT // 2]
                        mg = msg2p.tile([P, OUT], F32R, tag="mg2")
                        nc.vector.tensor_tensor(
                            out=mg[:],
                            in0=gsl.bitcast(F16),
                            in1=ew[:, k, None].to_broadcast((P, OUT)),
                            op=OP.mult)
                        nc.tensor.matmul(ps[:], lhsT=ident[:], rhs=mg[:],
                                         start=(k == 0), stop=(k == dd - 1))
                    ot = ep2p.tile([P, OUT], F32, tag="ot")
                    nc.vector.tensor_tensor(out=ot[:], in0=ps[:], in1=b2r[:],
                                            op=OP.add)
                    nc.sync.dma_start(
                        out_ap[:].rearrange("(i p) c -> p i c", p=P)[:, r, :],
                        ot[:])

    return kern


def declare_io(nc, in_maps, pr: Prob):
    ins_ap = {}
    for k, v in in_maps[0].items():
        if k in F32R_INPUTS:
            dt = F32R
        else:
            dt = mybir.dt.from_np(v.dtype)
        ins_ap[k] = nc.dram_tensor(
            f"in_{k}", list(v.shape), dt, kind="ExternalInput").ap()
    outs_ap = {"out": nc.dram_tensor(
        "out", [pr.NT, pr.OUT_DIM], F32, kind="ExternalOutput").ap()}
    return ins_ap, outs_ap


def assemble_output(results, sched, pr: Prob):
    out = np.zeros((pr.N, pr.OUT_DIM), np.float32)
    for c in range(NCC):
        nodes = sched["core_nodes"][c]
        oc = results[c]["out"]
        valid = nodes >= 0
        out[nodes[valid]] = oc[valid]
    return out


def kernel(**inputs) -> np.ndarray:
    pr = Prob(N=50000, IN_DIM=256, H1=8, HID=32, OUT_DIM=64)
    sched, in_maps = prep_all(inputs, pr)

    nc = bacc.Bacc("TRN2", target_bir_lowering=False, debug=False,
                   num_devices=NCC)
    ins_ap, outs_ap = declare_io(nc, in_maps, pr)
    kern = build_kernel_fn(pr)
    with tile.TileContext(nc) as tc:
        kern(tc, outs_ap, ins_ap)
    nc.compile()

    maps = [{f"in_{k}": v for k, v in m.items()} for m in in_maps]
    res = run_bass_kernel_spmd(nc, maps, core_ids=list(range(NCC)))
    return assemble_output(res.results, sched, pr)


def kernel_timed(inputs):
    import time
    pr = Prob(N=50000, IN_DIM=256, H1=8, HID=32, OUT_DIM=64)
    t0 = time.perf_counter()
    sched, in_maps = prep_all(inputs, pr)
    t1 = time.perf_counter()
    nc = bacc.Bacc("TRN2", target_bir_lowering=False, debug=False,
                   num_devices=NCC)
    ins_ap, outs_ap = declare_io(nc, in_maps, pr)
    kern = build_kernel_fn(pr)
    with tile.TileContext(nc) as tc:
        kern(tc, outs_ap, ins_ap)
    nc.compile()
    t2 = time.perf_counter()
    maps = [{f"in_{k}": v for k, v in m.items()} for m in in_maps]
    res = run_bass_kernel_spmd(nc, maps, core_ids=list(range(NCC)))
    t3 = time.perf_counter()
    res = run_bass_kernel_spmd(nc, maps, core_ids=list(range(NCC)))
    t4 = time.perf_counter()
    print(f"prep {t1-t0:.1f}s compile {t2-t1:.1f}s run1 {t3-t2:.2f}s "
          f"run2 {t4-t3:.2f}s (exec+IO)")
    return assemble_output(res.results, sched, pr)


if __name__ == "__main__":
    import pickle
    with open("/tmp/inputs.pkl", "rb") as f:
        inputs = pickle.load(f)
    out = kernel_timed(inputs)
    exp = np.load("/tmp/expected_np.npy")
    rel = np.linalg.norm(out - exp) / np.linalg.norm(exp)
    print("Relative error:", rel)

